# revision 23
# baseline (speedup 1.0000x reference)
# Trainium2 Bass kernel for CustomFullyConnectedLayer:
#   y = x @ W.T,  W[(c+i)%N, c] += V[i, c] for i in diag_pos  (banded weight)
# Strategy: data-parallel over batch across 8 cores. On each core:
#   y[b, r] = sum_{c in [r-29, r] mod N} x[b, c] * W[r, c]
# Tiled as 32 output blocks of 96 columns; each block needs a 128-wide
# (125 used) window of x features -> one K=128 matmul per block with a
# host-built band block of W.T. x windows are produced by PE transposes of
# a wrap-extended x tile (x_ext[:, k] = x[:, (k-32) mod N]).
import os
import sys

import numpy as np

if "/opt/trn_rl_repo" not in sys.path:
    sys.path.insert(0, "/opt/trn_rl_repo")

import ml_dtypes

BATCH = 8192
N = 3072
NCORES = 8
BC = BATCH // NCORES          # 1024 rows per core
NBT = BC // 128               # 8 batch tiles per core
RW = 96                       # output r-block width
NRB = N // RW                 # 32 r-blocks
PAD = 32                      # left extension of x (covers band offsets 0..29)

_CACHE = {}
LAST_RESULTS = None


def _build_program(dt_flag: str):
    import concourse.mybir as mybir
    import concourse.tile as tile
    from concourse import bacc

    cdt = mybir.dt.float32 if dt_flag == "fp32" else mybir.dt.bfloat16
    f32 = mybir.dt.float32
    ydt = mybir.dt.bfloat16 if dt_flag == "bf16y" else f32

    nc = bacc.Bacc("TRN2", target_bir_lowering=False, debug=False)
    xs = nc.dram_tensor("xs", [BC, N + PAD], cdt, kind="ExternalInput")
    wb = nc.dram_tensor("wb", [128, NRB, RW], cdt, kind="ExternalInput")
    ident = nc.dram_tensor("ident", [128, 128], cdt, kind="ExternalInput")
    ys = nc.dram_tensor("ys", [BC, N], ydt, kind="ExternalOutput")

    with tile.TileContext(nc) as tc:
        with (
            tc.tile_pool(name="consts", bufs=1) as consts,
            tc.tile_pool(name="xin", bufs=4) as xin,
            tc.tile_pool(name="xtp", bufs=3) as xtp,
            tc.tile_pool(name="yout", bufs=2) as yout,
            tc.tile_pool(name="ptr", bufs=3, space="PSUM") as ptr,
            tc.tile_pool(name="pyb", bufs=5, space="PSUM") as pyb,
        ):
            id_sb = consts.tile([128, 128], cdt)
            nc.sync.dma_start(out=id_sb, in_=ident[:, :])
            wb_sb = consts.tile([128, NRB, RW], cdt)
            nc.gpsimd.dma_start(out=wb_sb, in_=wb[:, :, :])

            # PE warm-up: dummy matmuls during the DMA fill so the HAM clock
            # gate opens (1.2 -> 2.4 GHz) before the first real transpose.
            wsrc = consts.tile([128, 128], cdt)
            nc.vector.memset(wsrc, 0.0)
            wps = pyb.tile([128, 5 * RW], f32, tag="py")
            for _ in range(56):
                nc.tensor.matmul(
                    wps[:, :128], lhsT=wsrc, rhs=wsrc, start=True, stop=True
                )

            xsplit = 1600  # transposes rho<16 need cols < 96*15+128 = 1568
            for t in range(NBT):
                rows = slice(t * 128, (t + 1) * 128)
                x_ext = xin.tile([128, N + PAD], cdt)
                if t == 0:
                    # finer split so the first transposes start sooner
                    nc.sync.dma_start(out=x_ext[:, :800], in_=xs[rows, :800])
                    nc.sync.dma_start(
                        out=x_ext[:, 800:xsplit], in_=xs[rows, 800:xsplit]
                    )
                else:
                    nc.sync.dma_start(out=x_ext[:, :xsplit], in_=xs[rows, :xsplit])
                nc.sync.dma_start(out=x_ext[:, xsplit:], in_=xs[rows, xsplit:])

                # transpose 32 feature windows: xT[p, b] = x_ext[b, 96*rho + p]
                tg = 8 if cdt == mybir.dt.bfloat16 else 4  # transposes per bank
                xT = xtp.tile([128, NRB, 128], cdt)
                for g in range(NRB // tg):
                    pt = ptr.tile([128, tg, 128], cdt)
                    for s in range(tg):
                        rho = tg * g + s
                        nc.tensor.transpose(
                            pt[:, s, :], x_ext[:, RW * rho: RW * rho + 128], id_sb
                        )
                    if cdt == mybir.dt.bfloat16:
                        # bitcast to int32: halves element count for the copy
                        nc.vector.tensor_copy(
                            out=xT[:, tg * g:tg * (g + 1), :].bitcast(
                                mybir.dt.int32
                            ),
                            in_=pt.bitcast(mybir.dt.int32),
                        )
                    else:
                        nc.vector.tensor_copy(
                            out=xT[:, tg * g:tg * (g + 1), :], in_=pt
                        )

                y_sb = yout.tile([128, N], ydt)
                for g in range(7):  # groups of <=5 r-blocks share a psum bank
                    w = min(5, NRB - 5 * g)
                    py = pyb.tile([128, 5 * RW], f32)
                    for k in range(w):
                        rho = 5 * g + k
                        nc.tensor.matmul(
                            py[:, RW * k: RW * (k + 1)],
                            lhsT=xT[:, rho, :],
                            rhs=wb_sb[:, rho, :],
                            start=True,
                            stop=True,
                        )
                    ydst = y_sb[:, 5 * RW * g: 5 * RW * g + w * RW]
                    if g == 3:
                        # balance copy load between ACT and DVE
                        nc.vector.tensor_copy(out=ydst, in_=py[:, : w * RW])
                    else:
                        nc.scalar.copy(out=ydst, in_=py[:, : w * RW])
                    if t == NBT - 1:
                        # last btile: store per group so the pipeline drain
                        # overlaps the final matmuls instead of serializing
                        nc.gpsimd.dma_start(
                            out=ys[rows, 5 * RW * g: 5 * RW * g + w * RW],
                            in_=y_sb[:, 5 * RW * g: 5 * RW * g + w * RW],
                        )
                if t < NBT - 1:
                    # single store per btile on the idle GPSIMD SWDGE queue,
                    # keeping SP free for x loads
                    nc.gpsimd.dma_start(out=ys[rows, :], in_=y_sb)

    nc.compile()
    return nc


def _host_prep(x, V, diag_pos, dt_flag):
    np_dt = np.float32 if dt_flag == "fp32" else ml_dtypes.bfloat16
    x = np.ascontiguousarray(np.asarray(x, dtype=np.float32))
    V = np.asarray(V, dtype=np.float32)
    diag = np.asarray(diag_pos).astype(np.int64) % N
    if diag.size and int(diag.max()) > PAD:
        raise ValueError(
            f"band kernel supports diag offsets <= {PAD}, got {int(diag.max())}"
        )

    # band[p, rho, q] = W.T[c, r] = W[r, c],  c=(RW*rho-PAD+p)%N, r=RW*rho+q
    # W[(c+i)%N, c] += V[i, c]  ->  band[q+PAD-i, rho, q] += V[i, (r-i)%N]
    band = np.zeros((128, NRB, RW), np.float32)
    rho = np.arange(NRB)[:, None]
    q = np.arange(RW)[None, :]
    for i in diag:
        i = int(i)
        c = (RW * rho + q - i) % N                     # [NRB, RW]
        p = q + PAD - i                                # [1, RW] in [3, 127]
        np.add.at(band, (np.broadcast_to(p, c.shape), rho, q), V[i, c])

    # x_ext[b, k] = x[b, (k - PAD) % N]
    x_ext = np.empty((BATCH, N + PAD), np_dt)
    x_ext[:, PAD:] = x
    x_ext[:, :PAD] = x[:, N - PAD:]

    band = band.astype(np_dt)
    identity = np.eye(128, dtype=np_dt)
    return x_ext, band, identity


def kernel(x, V, diag_pos):
    global LAST_RESULTS
    from concourse.bass_utils import run_bass_kernel_spmd

    dt_flag = os.environ.get("KERNEL_DTYPE", "bf16")
    if dt_flag not in _CACHE:
        _CACHE[dt_flag] = _build_program(dt_flag)
    nc = _CACHE[dt_flag]

    x_ext, band, identity = _host_prep(x, V, diag_pos, dt_flag)
    in_maps = [
        {
            "xs": x_ext[k * BC:(k + 1) * BC],
            "wb": band,
            "ident": identity,
        }
        for k in range(NCORES)
    ]
    res = run_bass_kernel_spmd(nc, in_maps, core_ids=list(range(NCORES)))
    LAST_RESULTS = res
    out = np.concatenate([r["ys"] for r in res.results], axis=0)
    return np.ascontiguousarray(out.astype(np.float32))


# revision 24
# speedup vs baseline: 1.0564x; 1.0564x over previous
# Trainium2 Bass kernel for CustomFullyConnectedLayer:
#   y = x @ W.T,  W[(c+i)%N, c] += V[i, c] for i in diag_pos  (banded weight)
# Strategy: data-parallel over batch across 8 cores. On each core:
#   y[b, r] = sum_{c in [r-29, r] mod N} x[b, c] * W[r, c]
# Tiled as 32 output blocks of 96 columns; each block needs a 128-wide
# (125 used) window of x features -> one K=128 matmul per block with a
# host-built band block of W.T. x windows are produced by PE transposes of
# a wrap-extended x tile (x_ext[:, k] = x[:, (k-32) mod N]).
import os
import sys

import numpy as np

if "/opt/trn_rl_repo" not in sys.path:
    sys.path.insert(0, "/opt/trn_rl_repo")

import ml_dtypes

BATCH = 8192
N = 3072
NCORES = 8
BC = BATCH // NCORES          # 1024 rows per core
NBT = BC // 128               # 8 batch tiles per core
RW = 96                       # output r-block width
NRB = N // RW                 # 32 r-blocks
PAD = 32                      # left extension of x (covers band offsets 0..29)

_CACHE = {}
LAST_RESULTS = None


def _build_program(dt_flag: str):
    import concourse.mybir as mybir
    import concourse.tile as tile
    from concourse import bacc

    cdt = mybir.dt.float32 if dt_flag == "fp32" else mybir.dt.bfloat16
    f32 = mybir.dt.float32
    ydt = mybir.dt.bfloat16 if dt_flag == "bf16y" else f32

    nc = bacc.Bacc("TRN2", target_bir_lowering=False, debug=False)
    xs = nc.dram_tensor("xs", [BC, N + PAD], cdt, kind="ExternalInput")
    wb = nc.dram_tensor("wb", [128, NRB, RW], cdt, kind="ExternalInput")
    ident = nc.dram_tensor("ident", [128, 128], cdt, kind="ExternalInput")
    ys = nc.dram_tensor("ys", [BC, N], ydt, kind="ExternalOutput")

    with tile.TileContext(nc) as tc:
        with (
            tc.tile_pool(name="consts", bufs=1) as consts,
            tc.tile_pool(name="xin", bufs=4) as xin,
            tc.tile_pool(name="xtp", bufs=3) as xtp,
            tc.tile_pool(name="yout", bufs=2) as yout,
            tc.tile_pool(name="ptr", bufs=3, space="PSUM") as ptr,
            tc.tile_pool(name="pyb", bufs=5, space="PSUM") as pyb,
        ):
            id_sb = consts.tile([128, 128], cdt)
            nc.sync.dma_start(out=id_sb, in_=ident[:, :])
            wb_sb = consts.tile([128, NRB, RW], cdt)
            nc.gpsimd.dma_start(out=wb_sb, in_=wb[:, :, :])

            # PE warm-up: dummy matmuls during the DMA fill so the HAM clock
            # gate opens (1.2 -> 2.4 GHz) before the first real transpose.
            wsrc = consts.tile([128, 128], cdt)
            nc.vector.memset(wsrc, 0.0)
            wps = pyb.tile([128, 5 * RW], f32, tag="py")
            for _ in range(56):
                nc.tensor.matmul(
                    wps[:, :128], lhsT=wsrc, rhs=wsrc, start=True, stop=True
                )

            xsplit = 1600  # transposes rho<16 need cols < 96*15+128 = 1568
            for t in range(NBT):
                rows = slice(t * 128, (t + 1) * 128)
                x_ext = xin.tile([128, N + PAD], cdt)
                if t == 0:
                    # finer split so the first transposes start sooner
                    nc.sync.dma_start(out=x_ext[:, :800], in_=xs[rows, :800])
                    nc.sync.dma_start(
                        out=x_ext[:, 800:xsplit], in_=xs[rows, 800:xsplit]
                    )
                else:
                    nc.sync.dma_start(out=x_ext[:, :xsplit], in_=xs[rows, :xsplit])
                nc.sync.dma_start(out=x_ext[:, xsplit:], in_=xs[rows, xsplit:])

                # transpose 32 feature windows: xT[p, b] = x_ext[b, 96*rho + p]
                tg = 8 if cdt == mybir.dt.bfloat16 else 4  # transposes per bank
                xT = xtp.tile([128, NRB, 128], cdt)
                for g in range(NRB // tg):
                    pt = ptr.tile([128, tg, 128], cdt)
                    for s in range(tg):
                        rho = tg * g + s
                        nc.tensor.transpose(
                            pt[:, s, :], x_ext[:, RW * rho: RW * rho + 128], id_sb
                        )
                    if cdt == mybir.dt.bfloat16:
                        # bitcast to int32: halves element count for the copy
                        nc.vector.tensor_copy(
                            out=xT[:, tg * g:tg * (g + 1), :].bitcast(
                                mybir.dt.int32
                            ),
                            in_=pt.bitcast(mybir.dt.int32),
                        )
                    else:
                        nc.vector.tensor_copy(
                            out=xT[:, tg * g:tg * (g + 1), :], in_=pt
                        )

                y_sb = yout.tile([128, N], ydt)
                for g in range(7):  # groups of <=5 r-blocks share a psum bank
                    w = min(5, NRB - 5 * g)
                    py = pyb.tile([128, 5 * RW], f32)
                    for k in range(w):
                        rho = 5 * g + k
                        nc.tensor.matmul(
                            py[:, RW * k: RW * (k + 1)],
                            lhsT=xT[:, rho, :],
                            rhs=wb_sb[:, rho, :],
                            start=True,
                            stop=True,
                        )
                    ydst = y_sb[:, 5 * RW * g: 5 * RW * g + w * RW]
                    if g == 0:
                        # balance copy load between ACT and DVE; group 0 is
                        # ready earliest so it can't head-of-line-block the
                        # next btile's xT copies in the DVE queue
                        nc.vector.tensor_copy(out=ydst, in_=py[:, : w * RW])
                    else:
                        nc.scalar.copy(out=ydst, in_=py[:, : w * RW])
                    if t == NBT - 1:
                        # last btile: store per group so the pipeline drain
                        # overlaps the final matmuls instead of serializing
                        nc.gpsimd.dma_start(
                            out=ys[rows, 5 * RW * g: 5 * RW * g + w * RW],
                            in_=y_sb[:, 5 * RW * g: 5 * RW * g + w * RW],
                        )
                if t < NBT - 1:
                    # single store per btile on the idle GPSIMD SWDGE queue,
                    # keeping SP free for x loads
                    nc.gpsimd.dma_start(out=ys[rows, :], in_=y_sb)

    nc.compile()
    return nc


def _host_prep(x, V, diag_pos, dt_flag):
    np_dt = np.float32 if dt_flag == "fp32" else ml_dtypes.bfloat16
    x = np.ascontiguousarray(np.asarray(x, dtype=np.float32))
    V = np.asarray(V, dtype=np.float32)
    diag = np.asarray(diag_pos).astype(np.int64) % N
    if diag.size and int(diag.max()) > PAD:
        raise ValueError(
            f"band kernel supports diag offsets <= {PAD}, got {int(diag.max())}"
        )

    # band[p, rho, q] = W.T[c, r] = W[r, c],  c=(RW*rho-PAD+p)%N, r=RW*rho+q
    # W[(c+i)%N, c] += V[i, c]  ->  band[q+PAD-i, rho, q] += V[i, (r-i)%N]
    band = np.zeros((128, NRB, RW), np.float32)
    rho = np.arange(NRB)[:, None]
    q = np.arange(RW)[None, :]
    for i in diag:
        i = int(i)
        c = (RW * rho + q - i) % N                     # [NRB, RW]
        p = q + PAD - i                                # [1, RW] in [3, 127]
        np.add.at(band, (np.broadcast_to(p, c.shape), rho, q), V[i, c])

    # x_ext[b, k] = x[b, (k - PAD) % N]
    x_ext = np.empty((BATCH, N + PAD), np_dt)
    x_ext[:, PAD:] = x
    x_ext[:, :PAD] = x[:, N - PAD:]

    band = band.astype(np_dt)
    identity = np.eye(128, dtype=np_dt)
    return x_ext, band, identity


def kernel(x, V, diag_pos):
    global LAST_RESULTS
    from concourse.bass_utils import run_bass_kernel_spmd

    dt_flag = os.environ.get("KERNEL_DTYPE", "bf16")
    if dt_flag not in _CACHE:
        _CACHE[dt_flag] = _build_program(dt_flag)
    nc = _CACHE[dt_flag]

    x_ext, band, identity = _host_prep(x, V, diag_pos, dt_flag)
    in_maps = [
        {
            "xs": x_ext[k * BC:(k + 1) * BC],
            "wb": band,
            "ident": identity,
        }
        for k in range(NCORES)
    ]
    res = run_bass_kernel_spmd(nc, in_maps, core_ids=list(range(NCORES)))
    LAST_RESULTS = res
    out = np.concatenate([r["ys"] for r in res.results], axis=0)
    return np.ascontiguousarray(out.astype(np.float32))
